# revision 22
# baseline (speedup 1.0000x reference)
"""Trainium2 Bass kernel for the GNN ExplainModule (masked adjacency).

Dense row-block design (8 NeuronCores, row-sharded output):
  - Core k owns rows [k*1250, (k+1)*1250). Rows are re-ordered by token
    count and grouped into 10 blocks of 125 rows (partitions 0-124).
  - Every mask contribution ("token") for cell (r, c) lives in the
    partition of its dest row r. Two streams per block share one slot
    grid of width S_b: stream1 = copy1 tokens (dest side uses W1a) on
    hidden partitions 0-63, stream2 = copy2 (dest side uses W1b) on
    partitions 64-127.
  - MLP runs in transposed layout [hidden x tokens]: host pre-gathers
    embed[c] columns (bf16) per token; PE computes (E @ W1x)^T per
    <=512-token chunk into a [128, L] pre tile (both streams stacked);
    the dest-row term (Eblk @ W1y + c)^T is added via a partition-run
    broadcast view; one relu covers both streams; a single PE matvec
    with the block-diagonal [w2;0|0;w2] weight yields both streams'
    logits [2, L]; a DRAM round-trip reshapes them into the fat
    [128, S] layout where the concrete gate is computed.
  - gpsimd local_scatter turns each block's gates into a dense
    [128, 2000] mask chunk; DVE multiplies by the adj chunk (bf16); the
    product is written out densely. Duplicate cells (same (r,c) fed by
    several edges) keep the first token in the dense path; the rare
    followers (~300/core) are applied afterwards with per-rank
    dma_scatter_add CCE adds of one-hot payloads.
"""

import sys

import numpy as np

for _p in ("/opt/trn_rl_repo",):
    if _p not in sys.path:
        sys.path.insert(0, _p)

import ml_dtypes

BF16 = ml_dtypes.bfloat16

N = 10000
D = 64
NCORES = 8
RPC = N // NCORES  # 1250 rows per core
NBLK = 10
RPB = RPC // NBLK  # 125 real rows per block
COLS = 10000
NCHUNK = 5
CHW = 2000  # dense chunk width


def _group_rank(key):
    """Rank of each element within its key-group (appearance order)."""
    o = np.argsort(key, kind="stable")
    ks = key[o]
    starts = np.flatnonzero(np.concatenate([[True], ks[1:] != ks[:-1]]))
    sizes = np.diff(np.concatenate([starts, [len(ks)]]))
    rank_sorted = np.arange(len(ks)) - np.repeat(starts, sizes)
    rank = np.empty(len(key), np.int64)
    rank[o] = rank_sorted
    return rank


def _prep_host(row, col, noise, adj, embed):
    """Route tokens, balance rows into blocks, build all per-core arrays."""
    row = np.asarray(row).astype(np.int64).ravel()
    col = np.asarray(col).astype(np.int64).ravel()
    noise = np.asarray(noise).astype(np.float32).ravel()
    adj = np.asarray(adj, dtype=np.float32)
    embed = np.asarray(embed, dtype=np.float32)
    embed_bf = embed.astype(BF16)

    E = row.shape[0]
    t_r = np.concatenate([row, col])  # dest row
    t_c = np.concatenate([col, row])  # dest col == other endpoint
    t_nz = np.concatenate([noise, noise])
    t_st = np.concatenate([np.zeros(E, np.int8), np.ones(E, np.int8)])
    core_of = t_r // RPC

    # ---- pass 1: per-core row stats and block structure ----
    per_core_tok = []
    orders = []
    Sm = np.zeros((NCORES, NBLK), np.int64)  # shared slot width per block
    for k in range(NCORES):
        m = core_of == k
        r_loc = (t_r[m] - k * RPC).astype(np.int64)
        cc = t_c[m].astype(np.int64)
        nz = t_nz[m]
        st = t_st[m].astype(np.int64)
        n1 = np.bincount(r_loc[st == 0], minlength=RPC)
        n2 = np.bincount(r_loc[st == 1], minlength=RPC)
        nm = np.maximum(n1, n2)
        order = np.argsort(-nm, kind="stable")
        orders.append(order)
        blk_of_row = np.empty(RPC, np.int64)
        part_of_row = np.empty(RPC, np.int64)
        for b in range(NBLK):
            rows_b = order[b * RPB : (b + 1) * RPB]
            blk_of_row[rows_b] = b
            part_of_row[rows_b] = np.arange(RPB)
            Sm[k, b] = max(int(nm[rows_b].max()), 1)
        per_core_tok.append((r_loc, cc, nz, st, blk_of_row, part_of_row))

    # SPMD-static shapes: max over cores
    Ss = Sm.max(axis=0)  # shared per-stream slot count per block
    SB = 2 * Ss  # fat width per block (even by construction)

    colb = np.concatenate([[0], np.cumsum(SB)]).astype(np.int64)
    SBT = int(colb[-1])
    oT = np.concatenate([[0], np.cumsum(128 * Ss)]).astype(np.int64)
    TT = int(oT[-1])  # per-stream token columns
    sidx_off = np.concatenate([[0], np.cumsum(NCHUNK * SB)]).astype(np.int64)

    # ---- pass 2: slots, duplicates, fixup ranks ----
    staged = []
    F = np.zeros((NCORES, NBLK), np.int64)
    for k in range(NCORES):
        r_loc, cc, nz, st, blk_of_row, part_of_row = per_core_tok[k]
        b_s = blk_of_row[r_loc]
        p_s = part_of_row[r_loc]
        key = ((b_s * 128 + p_s) * 2 + st) * N + cc
        o = np.argsort(key, kind="stable")
        b_s, p_s, c_s, nz_s, st_s = b_s[o], p_s[o], cc[o], nz[o], st[o]
        slot = _group_rank((b_s * 128 + p_s) * 2 + st_s)
        fat = np.where(st_s == 0, slot, Ss[b_s] + slot)
        crank = _group_rank((b_s * 128 + p_s) * N + c_s)
        is_fol = crank > 0
        frank = np.full(len(b_s), -1, np.int64)
        fi = np.flatnonzero(is_fol)
        if len(fi):
            frank[fi] = _group_rank(b_s[fi] * 128 + p_s[fi])
            for b in range(NBLK):
                mb = b_s[fi] == b
                F[k, b] = int(frank[fi][mb].max()) + 1 if mb.any() else 0
        staged.append(dict(b=b_s, p=p_s, c=c_s, nz=nz_s, st=st_s, fat=fat,
                           fol=fi, frank=frank))

    Fs = F.max(axis=0)
    foff = np.concatenate([[0], np.cumsum(Fs)]).astype(np.int64)
    fmoff = np.concatenate([[0], np.cumsum(Fs * SB)]).astype(np.int64)
    FT = max(int(foff[-1]), 1)
    FSB = max(int(fmoff[-1]), 1)
    NFX = max(int(Fs.sum()), 1)

    meta = dict(
        Ss=Ss, SB=SB, Fs=Fs, colb=colb, oT=oT,
        sidx_off=sidx_off, foff=foff, fmoff=fmoff,
        SBT=SBT, TT=TT, FT=FT, FSB=FSB, NFX=NFX,
    )

    bp_index = (
        np.repeat(np.arange(NBLK), RPB) * 128 + np.tile(np.arange(RPB), NBLK)
    )

    per_core = []
    for k in range(NCORES):
        s = staged[k]
        b_s, p_s, c_s, nz_s, st_s, fat = (
            s["b"], s["p"], s["c"], s["nz"], s["st"], s["fat"],
        )
        fi, frank = s["fol"], s["frank"]
        is_fol = np.zeros(len(b_s), bool)
        is_fol[fi] = True
        order = orders[k]

        egtc = np.zeros((128, TT), BF16)
        noisef = np.full((128, SBT), 0.5, np.float32)
        sidx = np.full((128, NCHUNK * SBT), -1, np.int16)
        fmask = np.zeros((128, FSB), BF16)
        leadm = np.zeros((128, FSB), np.float32)

        st1 = st_s == 0
        colx = oT[b_s] + p_s * Ss[b_s] + np.where(st1, fat, fat - Ss[b_s])
        egtc[:64, colx[st1]] = embed_bf[c_s[st1]].T
        egtc[64:, colx[~st1]] = embed_bf[c_s[~st1]].T
        noisef[p_s, colb[b_s] + fat] = nz_s
        keep = ~is_fol
        j = c_s // CHW
        sidx[
            p_s[keep],
            sidx_off[b_s[keep]] + j[keep] * SB[b_s[keep]] + fat[keep],
        ] = (c_s[keep] - j[keep] * CHW).astype(np.int16)

        if len(fi):
            # leader fat slot per cell group (first element in group order)
            cell = (b_s * 128 + p_s) * N + c_s
            co = np.argsort(cell, kind="stable")
            cs_ = cell[co]
            starts = np.flatnonzero(
                np.concatenate([[True], cs_[1:] != cs_[:-1]])
            )
            sizes = np.diff(np.concatenate([starts, [len(cs_)]]))
            lead_fat = np.empty(len(cell), np.int64)
            lead_fat[co] = np.repeat(fat[co][starts], sizes)
            fb, fp, fr = b_s[fi], p_s[fi], frank[fi]
            fmask[fp, fmoff[fb] + fr * SB[fb] + fat[fi]] = 1
            leadm[fp, fmoff[fb] + fr * SB[fb] + lead_fat[fi]] = 1.0

        adjp = np.zeros((NBLK * 128, COLS), BF16)
        embp = np.zeros((NBLK * 128, 64), np.float32)
        rows_g = order + k * RPC
        adjp[bp_index, :N] = adj[rows_g].astype(BF16)
        embp[bp_index] = embed[rows_g]

        per_core.append(
            dict(
                egtc=egtc, noisef=noisef, sidx=sidx,
                fmask=fmask, leadm=leadm, adjp=adjp, embp=embp,
            )
        )
    return per_core, orders, meta


def _emulate_core(m, meta, W1, b1, W2, b2):
    """Numpy emulation of the device program for one core (testing aid)."""
    Ss, SB = meta["Ss"], meta["SB"]
    Fs, colb, oT = meta["Fs"], meta["colb"], meta["oT"]
    sidx_off, foff, fmoff = meta["sidx_off"], meta["foff"], meta["fmoff"]

    W1a = W1[0:64].astype(np.float32)
    W1b = W1[64:128].astype(np.float32)
    w2 = W2.reshape(-1).astype(BF16).astype(np.float32)
    W1ab = W1a.astype(BF16).astype(np.float32)
    W1bb = W1b.astype(BF16).astype(np.float32)
    crow = m["_crow"]

    out = np.zeros((NBLK * 128, COLS), np.float32)
    egt1 = m["egtc"][:64].astype(np.float32)
    egt2 = m["egtc"][64:].astype(np.float32)
    embp = m["embp"]

    sfat = np.zeros((128, meta["SBT"]), np.float32)
    for b in range(NBLK):
        Eblk = embp[b * 128 : (b + 1) * 128]
        PAT = (Eblk @ W1a + crow).T
        PBT = (Eblk @ W1b + crow).T
        S_b = Ss[b]
        L = 128 * S_b
        pre1 = W1bb.T @ egt1[:, oT[b] : oT[b] + L] + np.repeat(PAT, S_b, axis=1)
        pre2 = W1ab.T @ egt2[:, oT[b] : oT[b] + L] + np.repeat(PBT, S_b, axis=1)
        pre1 = np.maximum(pre1.astype(BF16).astype(np.float32), 0.0)
        pre2 = np.maximum(pre2.astype(BF16).astype(np.float32), 0.0)
        s1 = (w2 @ pre1).reshape(128, S_b)
        s2 = (w2 @ pre2).reshape(128, S_b)
        sfat[:, colb[b] : colb[b] + S_b] = s1
        sfat[:, colb[b] + S_b : colb[b] + SB[b]] = s2

    nz = m["noisef"]
    z = np.log(nz) - np.log1p(-nz) + sfat + float(b2)
    gate = 1.0 / (1.0 + np.exp(-z))
    gatebf = (gate * 0.5).astype(BF16)

    for b in range(NBLK):
        gsl = gatebf[:, colb[b] : colb[b] + SB[b]]
        for r in range(Fs[b]):
            fm = m["fmask"][:, fmoff[b] + r * SB[b] : fmoff[b] + (r + 1) * SB[b]]
            lm = m["leadm"][:, fmoff[b] + r * SB[b] : fmoff[b] + (r + 1) * SB[b]]
            famt = (gsl.astype(np.float32) * fm.astype(np.float32)).sum(
                axis=1, keepdims=True
            )
            tl = (famt * lm).astype(BF16)
            gsl[:] = (gsl.astype(np.float32) + tl.astype(np.float32)).astype(
                BF16
            )
        for j in range(NCHUNK):
            idx = m["sidx"][
                :, sidx_off[b] + j * SB[b] : sidx_off[b] + (j + 1) * SB[b]
            ]
            mask = np.zeros((128, CHW), BF16)
            rows, cols_ = np.where(idx >= 0)
            mask[rows, idx[rows, cols_]] = gsl[rows, cols_]
            prod = (
                m["adjp"][b * 128 : (b + 1) * 128, j * CHW : (j + 1) * CHW]
                * mask
            ).astype(BF16)
            out[b * 128 : (b + 1) * 128, j * CHW : (j + 1) * CHW] = prod
    return out


def _build_program(meta, b2f):
    import concourse.bacc as bacc
    import concourse.mybir as mybir
    import concourse.tile as tile
    from concourse.masks import make_identity

    f32 = mybir.dt.float32
    bf16 = mybir.dt.bfloat16
    i16 = mybir.dt.int16
    add = mybir.AluOpType.add
    mult = mybir.AluOpType.mult
    subtract = mybir.AluOpType.subtract
    is_equal = mybir.AluOpType.is_equal
    AF = mybir.ActivationFunctionType

    Ss, SB = meta["Ss"], meta["SB"]
    Fs, colb, oT = meta["Fs"], meta["colb"], meta["oT"]
    sidx_off, foff, fmoff = meta["sidx_off"], meta["foff"], meta["fmoff"]
    SBT, TT, FT, FSB, NFX = (
        meta["SBT"], meta["TT"], meta["FT"], meta["FSB"], meta["NFX"],
    )
    have_fx = int(Fs.sum()) > 0
    LMAX = 128 * int(Ss.max())

    nc = bacc.Bacc()

    egtcp = nc.declare_dram_parameter("egtc", [128, TT], bf16, isOutput=False)
    noisep = nc.declare_dram_parameter("noisef", [128, SBT], f32, isOutput=False)
    sidxp = nc.declare_dram_parameter("sidx", [128, NCHUNK * SBT], i16, isOutput=False)
    fmaskp = nc.declare_dram_parameter("fmask", [128, FSB], bf16, isOutput=False)
    leadmp = nc.declare_dram_parameter("leadm", [128, FSB], f32, isOutput=False)
    adjp = nc.declare_dram_parameter("adjp", [NBLK * 128, COLS], bf16, isOutput=False)
    embp = nc.declare_dram_parameter("embp", [NBLK * 128, 64], f32, isOutput=False)
    e5p = nc.declare_dram_parameter("e5", [64, 1], f32, isOutput=False)
    w1afp = nc.declare_dram_parameter("w1af", [64, 64], f32, isOutput=False)
    w1bfp = nc.declare_dram_parameter("w1bf", [64, 64], f32, isOutput=False)
    w1cfp = nc.declare_dram_parameter("w1cf", [64, 64], f32, isOutput=False)
    wbigp = nc.declare_dram_parameter("wbig", [128, 128], bf16, isOutput=False)
    w2dp = nc.declare_dram_parameter("w2d", [128, 2], bf16, isOutput=False)
    b1rp = nc.declare_dram_parameter("b1r", [1, 64], f32, isOutput=False)
    outp = nc.declare_dram_parameter("out", [NBLK * 128, COLS], bf16, isOutput=True)

    sdram = nc.dram_tensor("sdram", [2 * NBLK, LMAX], bf16)

    with tile.TileContext(nc) as tc:
        with (
            tc.tile_pool(name="const", bufs=1) as cp,
            tc.tile_pool(name="blk", bufs=2) as bp,
            tc.tile_pool(name="srowp", bufs=1) as srp,
            tc.tile_pool(name="work", bufs=3) as wp,
            tc.tile_pool(name="bigio", bufs=1) as bio,
            tc.tile_pool(name="small", bufs=2) as sp,
            tc.tile_pool(name="psA", bufs=4, space="PSUM") as ppA,
            tc.tile_pool(name="psB", bufs=2, space="PSUM") as ppB,
            tc.tile_pool(name="psC", bufs=1, space="PSUM") as ppC,
        ):
            b0 = NBLK - 1
            et0 = bp.tile([128, 64], f32, tag="et")
            nc.sync.dma_start(out=et0[:], in_=embp[b0 * 128 : (b0 + 1) * 128, :])
            egtc0 = bp.tile([128, 128 * int(Ss[b0])], bf16, tag="egtc")
            nc.sync.dma_start(
                out=egtc0[:],
                in_=egtcp[:, int(oT[b0]) : int(oT[b0]) + 128 * int(Ss[b0])],
            )
            identity = cp.tile([128, 128], f32)
            make_identity(nc, identity[:])
            w1af = cp.tile([64, 64], f32)
            nc.sync.dma_start(out=w1af[:], in_=w1afp[:, :])
            w1bf = cp.tile([64, 64], f32)
            nc.sync.dma_start(out=w1bf[:], in_=w1bfp[:, :])
            w1cf = cp.tile([64, 64], f32)
            nc.sync.dma_start(out=w1cf[:], in_=w1cfp[:, :])
            wbig = cp.tile([128, 128], bf16)
            nc.sync.dma_start(out=wbig[:], in_=wbigp[:, :])
            w2d = cp.tile([128, 2], bf16)
            nc.sync.dma_start(out=w2d[:], in_=w2dp[:, :])
            b1t = cp.tile([1, 64], f32)
            nc.sync.dma_start(out=b1t[:], in_=b1rp[:, :])
            e5t = cp.tile([64, 1], f32)
            nc.sync.dma_start(out=e5t[:], in_=e5p[:, :])
            ones128 = cp.tile([1, 128], f32)
            nc.vector.memset(ones128[:], 1.0)
            noiset = cp.tile([128, SBT], f32)
            nc.sync.dma_start(out=noiset[:], in_=noisep[:, :])
            sidxt = cp.tile([128, NCHUNK * SBT], i16)
            nc.sync.dma_start(out=sidxt[:], in_=sidxp[:, :])
            if have_fx:
                fmaskt = cp.tile([128, FSB], bf16)
                nc.sync.dma_start(out=fmaskt[:], in_=fmaskp[:, :])
                leadmt = cp.tile([128, FSB], f32)
                nc.sync.dma_start(out=leadmt[:], in_=leadmp[:, :])
            sfatb = cp.tile([128, SBT], bf16)
            gatebf = cp.tile([128, SBT], bf16)
            zpre = cp.tile([128, SBT], f32)

            cps = ppC.tile([1, 64], f32, tag="cps")
            nc.tensor.matmul(cps[:], lhsT=e5t[:], rhs=w1cf[:], start=True, stop=True)
            crow = cp.tile([1, 64], f32)
            nc.vector.tensor_tensor(out=crow[:], in0=cps[:], in1=b1t[:], op=add)

            # noise logit for all blocks: zpre = ln(nz) + b2 - ln(1 - nz)
            nc.vector.tensor_scalar(
                out=zpre[:], in0=noiset[:], scalar1=-1.0, scalar2=1.0,
                op0=mult, op1=add,
            )
            nc.scalar.activation(out=zpre[:], in_=zpre[:], func=AF.Ln)
            nc.scalar.activation(out=noiset[:], in_=noiset[:], func=AF.Ln)
            nc.vector.scalar_tensor_tensor(
                out=zpre[:], in0=noiset[:], scalar=b2f, in1=zpre[:],
                op0=add, op1=subtract,
            )

            def load_block(b):
                S_b = int(Ss[b])
                L = 128 * S_b
                oo = int(oT[b])
                et = bp.tile([128, 64], f32, tag="et")
                nc.sync.dma_start(out=et[:], in_=embp[b * 128 : (b + 1) * 128, :])
                egtc = bp.tile([128, L], bf16, tag="egtc")
                nc.sync.dma_start(out=egtc[:], in_=egtcp[:, oo : oo + L])
                return et, egtc

            def mlp_block(b, et, egtc):
                # dest-side tables PAT/PBT stacked [128, 128] = (Eblk @ W1x + c)^T
                tps = ppC.tile([64, 128], f32, tag="blkps")
                nc.tensor.transpose(tps[:, :], et[:, :], identity[:, :])
                ebT = bp.tile([64, 128], f32, tag="ebT")
                nc.scalar.copy(out=ebT[:], in_=tps[:])
                patpbt = bp.tile([128, 128], f32, tag="patpbt")
                patp = ppC.tile([64, 128], f32, tag="blkps")
                nc.tensor.matmul(patp[:], lhsT=w1af[:], rhs=ebT[:], start=True, stop=False)
                nc.tensor.matmul(patp[:], lhsT=crow[:], rhs=ones128[:], start=False, stop=True)
                nc.scalar.copy(out=patpbt[0:64, :], in_=patp[:])
                pbtp = ppC.tile([64, 128], f32, tag="blkps")
                nc.tensor.matmul(pbtp[:], lhsT=w1bf[:], rhs=ebT[:], start=True, stop=False)
                nc.tensor.matmul(pbtp[:], lhsT=crow[:], rhs=ones128[:], start=False, stop=True)
                nc.scalar.copy(out=patpbt[64:128, :], in_=pbtp[:])

                S_b = int(Ss[b])
                L = 128 * S_b
                g = max(min(512 // S_b, 128), 1)

                pre = bp.tile([128, L], bf16, tag="pre")
                p0 = 0
                while p0 < 128:
                    gg = min(g, 128 - p0)
                    Lc = gg * S_b
                    c0 = p0 * S_b
                    pps = ppA.tile([128, Lc], f32, tag="pps")
                    nc.tensor.matmul(
                        pps[:], lhsT=wbig[:], rhs=egtc[:, c0 : c0 + Lc],
                        start=True, stop=True,
                    )
                    pt_b = (
                        patpbt[:, p0 : p0 + gg]
                        .rearrange("h (g o) -> h g o", o=1)
                        .to_broadcast([128, gg, S_b])
                    )
                    nc.vector.tensor_tensor(
                        out=pre[:, c0 : c0 + Lc].rearrange(
                            "h (g s) -> h g s", s=S_b
                        ),
                        in0=pps[:].rearrange("h (g s) -> h g s", s=S_b),
                        in1=pt_b,
                        op=add,
                    )
                    p0 += gg
                nc.scalar.activation(out=pre[:], in_=pre[:], func=AF.Relu)
                srow = srp.tile([2, L], bf16, tag="srow")
                p0 = 0
                while p0 < 128:
                    gg = min(g, 128 - p0)
                    Lc = gg * S_b
                    c0 = p0 * S_b
                    sps = ppB.tile([2, Lc], f32, tag="sps")
                    nc.tensor.matmul(
                        sps[:], lhsT=w2d[:], rhs=pre[:, c0 : c0 + Lc],
                        start=True, stop=True,
                    )
                    nc.scalar.copy(out=srow[:, c0 : c0 + Lc], in_=sps[:])
                    p0 += gg
                # DRAM round-trip reshape [2, L] -> two [128, S_b] halves
                nc.sync.dma_start(out=sdram[2 * b : 2 * b + 2, 0:L], in_=srow[:])
                cdst = int(colb[b])
                for st in (0, 1):
                    nc.sync.dma_start(
                        out=sfatb[:, cdst + st * S_b : cdst + st * S_b + S_b],
                        in_=sdram[2 * b + st : 2 * b + st + 1, 0:L].rearrange(
                            "o (p s) -> (o p) s", p=128
                        ),
                    )

                # gate math on fat slice [128, SB_b]
                sb = int(SB[b])
                c0 = int(colb[b])
                z = sp.tile([128, sb], f32, tag="z")
                nc.vector.tensor_scalar_add(z[:], sfatb[:, c0 : c0 + sb], 0.0)
                nc.vector.tensor_tensor(
                    out=z[:], in0=z[:], in1=zpre[:, c0 : c0 + sb], op=add
                )
                gf = sp.tile([128, sb], f32, tag="gf")
                nc.scalar.activation(out=gf[:], in_=z[:], func=AF.Sigmoid)
                nc.vector.tensor_scalar_mul(gatebf[:, c0 : c0 + sb], gf[:], 0.5)

                # fold duplicate-cell follower gates into their leader slot
                for r in range(int(Fs[b])):
                    fsl = slice(int(fmoff[b]) + r * sb, int(fmoff[b]) + (r + 1) * sb)
                    prod = sp.tile([128, sb], bf16, tag="fprod")
                    nc.vector.tensor_tensor(
                        out=prod[:], in0=gatebf[:, c0 : c0 + sb],
                        in1=fmaskt[:, fsl], op=mult,
                    )
                    famt = sp.tile([128, 1], f32, tag="famt")
                    nc.vector.tensor_reduce(
                        out=famt[:], in_=prod[:], axis=mybir.AxisListType.X,
                        op=add,
                    )
                    tl = sp.tile([128, sb], bf16, tag="tl")
                    nc.vector.tensor_tensor(
                        out=tl[:], in0=leadmt[:, fsl],
                        in1=famt[:].to_broadcast([128, sb]), op=mult,
                    )
                    nc.vector.tensor_tensor(
                        out=gatebf[:, c0 : c0 + sb],
                        in0=gatebf[:, c0 : c0 + sb], in1=tl[:], op=add,
                    )

            def prefetch_adj(b):
                adjt = bio.tile([128, COLS], bf16, tag="adjt")
                nc.sync.dma_start(
                    out=adjt[:], in_=adjp[b * 128 : (b + 1) * 128, :]
                )
                return adjt

            def dense_block(b, adjt):
                sb = int(SB[b])
                c0 = int(colb[b])
                outt = bio.tile([128, COLS], bf16, tag="outt")
                for j in range(NCHUNK):
                    mask = wp.tile([128, CHW], bf16, tag="mask")
                    nc.gpsimd.local_scatter(
                        out_ap=mask[:],
                        data_ap=gatebf[:, c0 : c0 + sb],
                        idxs_ap=sidxt[
                            :,
                            int(sidx_off[b]) + j * sb : int(sidx_off[b])
                            + (j + 1) * sb,
                        ],
                        channels=128,
                        num_elems=CHW,
                        num_idxs=sb,
                    )
                    nc.vector.tensor_tensor(
                        out=outt[:, j * CHW : (j + 1) * CHW], in0=mask[:],
                        in1=adjt[:, j * CHW : (j + 1) * CHW], op=mult,
                    )
                nc.sync.dma_start(
                    out=outp[b * 128 : (b + 1) * 128, :], in_=outt[:]
                )

            # software pipeline: prefetch inputs one block ahead; the dense
            # phase of the previous block overlaps the MLP of the current
            # one. Process smallest blocks first so the un-overlapped
            # startup MLP is short.
            PROC = list(range(NBLK - 1, -1, -1))
            held = (et0, egtc0)
            adj_held = None
            for i, b in enumerate(PROC):
                et, egtc = held
                if i > 0:
                    adj_held = prefetch_adj(PROC[i - 1])
                if i == NBLK - 1:
                    # last block: emit the previous dense phase first so it
                    # overlaps this (longest) MLP instead of trailing it
                    dense_block(PROC[i - 1], adj_held)
                mlp_block(b, et, egtc)
                if i + 1 < NBLK:
                    held = load_block(PROC[i + 1])
                if 0 < i < NBLK - 1:
                    dense_block(PROC[i - 1], adj_held)
            adj_held = prefetch_adj(PROC[-1])
            dense_block(PROC[-1], adj_held)

    nc.compile()
    return nc


def kernel(embed, row, col, adj, noise, W1, b1, W2, b2, node_idx):
    from concourse.bass_utils import run_bass_kernel_spmd

    embed = np.ascontiguousarray(np.asarray(embed), dtype=np.float32)
    adj = np.ascontiguousarray(np.asarray(adj), dtype=np.float32)
    W1 = np.ascontiguousarray(np.asarray(W1), dtype=np.float32)
    b1 = np.ascontiguousarray(np.asarray(b1), dtype=np.float32).ravel()
    W2 = np.ascontiguousarray(np.asarray(W2), dtype=np.float32)
    b2f = float(np.asarray(b2, dtype=np.float32).ravel()[0])
    nidx = int(np.asarray(node_idx))

    per_core, orders, meta = _prep_host(row, col, noise, adj, embed)
    nc = _build_program(meta, b2f)

    w1a = np.ascontiguousarray(W1[0:64])
    w1b = np.ascontiguousarray(W1[64:128])
    w1c = np.ascontiguousarray(W1[128:192])
    w2v = W2.reshape(-1)
    w2d = np.zeros((128, 2), np.float32)
    w2d[:64, 0] = w2v
    w2d[64:, 1] = w2v
    wbig = np.zeros((128, 128), np.float32)
    wbig[0:64, 0:64] = w1b  # stream1 other-side
    wbig[64:128, 64:128] = w1a  # stream2 other-side
    common = dict(
        e5=np.ascontiguousarray(embed[nidx].reshape(64, 1)),
        w1af=w1a, w1bf=w1b, w1cf=w1c,
        wbig=np.ascontiguousarray(wbig.astype(BF16)),
        w2d=np.ascontiguousarray(w2d.astype(BF16)),
        b1r=np.ascontiguousarray(b1.reshape(1, 64)),
    )
    in_maps = []
    for k in range(NCORES):
        mcore = dict(per_core[k])
        mcore.update(common)
        in_maps.append(mcore)

    res = run_bass_kernel_spmd(nc, in_maps, list(range(NCORES)))
    kernel.last_exec_time_ns = res.exec_time_ns
    it = getattr(res, "instructions_and_trace", None)
    kernel.last_trace_path = it[1] if it else None

    bp_index = (
        np.repeat(np.arange(NBLK), RPB) * 128 + np.tile(np.arange(RPB), NBLK)
    )
    out = np.empty((N, N), np.float32)
    for k in range(NCORES):
        o = np.asarray(res.results[k]["out"])[:, :N].astype(np.float32)
        out[orders[k] + k * RPC] = o[bp_index]
    return out


kernel.last_exec_time_ns = None
kernel.last_trace_path = None
